# revision 9
# baseline (speedup 1.0000x reference)
"""Trainium2 Bass kernel: causal multi-head self-attention, last-position output.

Reference computes full causal MHSA on x[B=8, S=2048, F=256] and returns only
out[:, -1, :].  The last query row attends to every position unmasked, so the
whole problem collapses per batch element to:

    q_row  = x[-1] @ Wq                              [512]      (host)
    M[f,h] = sum_{d} Wk[f, h*64+d] * q_row[h*64+d]   [256, 8]   (host)
    scores = (x @ M).T                               [8, 2048]
    w      = exp(scores / 8)  (softmax, no max-sub: |scores/8| < 5.1)
    attn_x = (w / w.sum(-1)) @ x                     [8, 256]
    attn_f = attn_x @ Wv                             [8, 512]
    a[p]   = attn_f[p // 64, p]                      [512]   (block-diag extract)
    out    = a @ Wo + bo                             [256]

The big x@Wk / x@Wv matmuls vanish via associativity.  q_row and M depend only
on one x row + weights (0.26 MFLOP) and are folded on the host, which also
removes the Wq/Wk transfers.  Per-core device cost: read x (2MB) + Wv/Wo
(1MB), ~45 matmuls, fully pipelined per 512-row group.  Sharding: pure data
parallel over batch, core b <- batch b.  No collectives.
"""

import numpy as np
from contextlib import ExitStack

import concourse.bass as bass
import concourse.tile as tile
from concourse import bacc, mybir
from concourse.bass_utils import run_bass_kernel_spmd
from concourse.masks import make_identity

B, S, F, PROJ, H, D = 8, 2048, 256, 512, 8, 64
NT = S // 128        # 16 s-tiles
FC = F // 128        # 2 f-chunks
SG = 4               # s-tiles per pipeline group
NG = NT // SG        # 4 groups
f32 = mybir.dt.float32
f32r = mybir.dt.float32r
EXP = mybir.ActivationFunctionType.Exp

_cache = {}


def _build():
    nc = bacc.Bacc("TRN2", target_bir_lowering=False, debug=False, num_devices=B)
    x = nc.dram_tensor("x", [S, F], f32, kind="ExternalInput").ap()
    M = nc.dram_tensor("M", [F, H], f32, kind="ExternalInput").ap()
    Wv = nc.dram_tensor("Wv", [F, PROJ], f32, kind="ExternalInput").ap()
    Wo = nc.dram_tensor("Wo", [PROJ, F], f32, kind="ExternalInput").ap()
    bo = nc.dram_tensor("bo", [F], f32, kind="ExternalInput").ap()
    out = nc.dram_tensor("out", [F], f32, kind="ExternalOutput").ap()

    with tile.TileContext(nc) as tc, ExitStack() as ctx:
        P = ctx.enter_context(tc.tile_pool(name="persist", bufs=1))
        xtp = ctx.enter_context(tc.tile_pool(name="xtp", bufs=3, space="PSUM"))
        scp = ctx.enter_context(tc.tile_pool(name="scp", bufs=2, space="PSUM"))
        smp = ctx.enter_context(tc.tile_pool(name="smp", bufs=3, space="PSUM"))

        ident = P.tile([128, 128], f32)
        x_sb = P.tile([128, NT, F], f32)
        xT_sb = P.tile([128, FC, S], f32)
        m_sb = P.tile([128, FC, H], f32)
        wv_sb = P.tile([128, FC, PROJ], f32)
        wo_sb = P.tile([128, 4, F], f32)
        bo_sb = P.tile([128, FC], f32)
        w_sb = P.tile([H, S], f32)
        wt_sb = P.tile([128, NT * H], f32)
        ssum4 = P.tile([H, NG], f32)
        ssum = P.tile([H, 1], f32)
        srecip = P.tile([H, 1], f32)
        axn_sb = P.tile([H, F], f32)
        axT_sb = P.tile([128, FC * H], f32)
        af_sb = P.tile([H, PROJ], f32)
        ac_sb = P.tile([128, 4], f32)
        o_sb = P.tile([128, FC], f32)
        dummy = P.tile([1, 1], f32)

        # trigger the ACT Exp table load early, overlapped with DMA
        nc.vector.memset(dummy[:], 0.0)
        nc.scalar.activation(out=dummy[:], in_=dummy[:], func=EXP)

        make_identity(nc, ident[:])

        # ---- DMAs: x group 0 in halves (earlier compute start), rest of x,
        #      tiny M between, tail weights
        xr = x.rearrange("(t p) f -> p t f", p=128)
        nc.sync.dma_start(out=x_sb[:, 0:2, :], in_=xr[:, 0:2, :])
        nc.sync.dma_start(out=x_sb[:, 2:SG, :], in_=xr[:, 2:SG, :])
        nc.sync.dma_start(out=x_sb[:, SG : 2 * SG, :], in_=xr[:, SG : 2 * SG, :])
        nc.sync.dma_start(out=m_sb[:], in_=M.rearrange("(c p) h -> p c h", p=128))
        for g in range(2, NG):
            nc.sync.dma_start(
                out=x_sb[:, g * SG : (g + 1) * SG, :], in_=xr[:, g * SG : (g + 1) * SG, :]
            )
        nc.sync.dma_start(out=wv_sb[:], in_=Wv.rearrange("(c p) n -> p c n", p=128))
        nc.sync.dma_start(out=wo_sb[:], in_=Wo.rearrange("(c p) n -> p c n", p=128))
        nc.sync.dma_start(out=bo_sb[:], in_=bo.rearrange("(c p) -> p c", p=128))

        # ---- PE warm-up: dummy transposes while DMA streams, so the HAM
        #      clock gate opens (1.2 -> 2.4 GHz) before the real work arrives
        warm_ps = xtp.tile([128, SG * 128], f32, tag="xt")
        for j in range(8):
            nc.tensor.transpose(
                warm_ps[:, (j % SG) * 128 : (j % SG + 1) * 128], ident[:], ident[:]
            )

        # ---- pipelined per 512-row group:
        #      PE transpose x -> DVE copy -> PE scores -> ACT exp -> PE w-transpose
        #      -> DVE copy -> PE attn accumulate
        ax_ps = None
        for g in range(NG):
            lo, hi = g * SG * 128, (g + 1) * SG * 128
            for c in range(FC):
                xt_ps = xtp.tile([128, SG * 128], f32, tag="xt")
                for j in range(SG):
                    nc.tensor.transpose(
                        xt_ps[:, j * 128 : (j + 1) * 128],
                        x_sb[:, g * SG + j, c * 128 : (c + 1) * 128],
                        ident[:],
                    )
                nc.vector.tensor_copy(xT_sb[:, c, lo:hi], xt_ps[:])
            sc_ps = scp.tile([H, SG * 128], f32, tag="sc")
            for c in range(FC):
                nc.tensor.matmul(
                    sc_ps[:],
                    m_sb[:, c, :],
                    xT_sb[:, c, lo:hi],
                    start=(c == 0),
                    stop=(c == FC - 1),
                )
            nc.scalar.activation(
                out=w_sb[:, lo:hi],
                in_=sc_ps[:],
                func=EXP,
                scale=0.125,
                accum_out=ssum4[:, g : g + 1],
            )
            wt_ps = smp.tile([128, SG * H], f32, tag="sm")
            for j in range(SG):
                t_idx = g * SG + j
                nc.tensor.transpose(
                    wt_ps[:, j * H : (j + 1) * H],
                    w_sb[:, t_idx * 128 : (t_idx + 1) * 128],
                    ident[:H, :H],
                )
            nc.vector.tensor_copy(wt_sb[:, g * SG * H : (g + 1) * SG * H], wt_ps[:])
            if ax_ps is None:
                ax_ps = smp.tile([H, F], f32, tag="sm")
            for j in range(SG):
                t_idx = g * SG + j
                nc.tensor.matmul(
                    ax_ps[:],
                    wt_sb[:, t_idx * H : (t_idx + 1) * H],
                    x_sb[:, t_idx, :],
                    start=(t_idx == 0),
                    stop=(t_idx == NT - 1),
                )

        # ---- softmax denominator + normalize
        nc.vector.reduce_sum(out=ssum[:], in_=ssum4[:], axis=mybir.AxisListType.X)
        nc.vector.reciprocal(srecip[:], ssum[:])
        nc.vector.tensor_scalar_mul(axn_sb[:], ax_ps[:], srecip[:])

        # ---- attn_x.T chunks [f-part, h]
        axt_ps = smp.tile([128, FC * H], f32, tag="sm")
        for c in range(FC):
            nc.tensor.transpose(
                axt_ps[:, c * H : (c + 1) * H],
                axn_sb[:, c * 128 : (c + 1) * 128],
                ident[:H, :H],
            )
        nc.vector.tensor_copy(axT_sb[:], axt_ps[:])

        # ---- attn_full[8, 512] = attn_x @ Wv
        af_ps = smp.tile([H, PROJ], f32, tag="sm")
        for c in range(FC):
            nc.tensor.matmul(
                af_ps[:],
                axT_sb[:, c * H : (c + 1) * H],
                wv_sb[:, c, :],
                start=(c == 0),
                stop=(c == FC - 1),
            )
        nc.vector.tensor_copy(af_sb[:, 0:256], af_ps[:, 0:256])
        nc.scalar.copy(af_sb[:, 256:512], af_ps[:, 256:512])

        # ---- block-diagonal extract -> attn_col [128, 4]  (col c = PROJ chunk c)
        aft_ps = smp.tile([128, 4 * H], f32, tag="sm")
        for c in range(4):
            nc.tensor.transpose(
                aft_ps[:, c * H : (c + 1) * H],
                af_sb[:, c * 128 : (c + 1) * 128],
                ident[:H, :H],
            )
        # aft[j, 8c+h] = attn_f[h, 128c+j]; want col 10c + (j>=64) per chunk c
        top = aft_ps[0:64, 0:1]
        bot = aft_ps[64:128, 1:2]
        nc.vector.tensor_copy(
            ac_sb[0:64, 0:4], bass.AP(tensor=top.tensor, offset=top.offset, ap=[top.ap[0], [10, 4]])
        )
        nc.vector.tensor_copy(
            ac_sb[64:128, 0:4], bass.AP(tensor=bot.tensor, offset=bot.offset, ap=[bot.ap[0], [10, 4]])
        )

        # ---- out[256] = attn_col.T @ Wo + bo  (column layout [128, 2])
        o_ps = smp.tile([128, FC], f32, tag="sm")
        for mc in range(FC):
            for c in range(4):
                nc.tensor.matmul(
                    o_ps[:, mc : mc + 1],
                    wo_sb[:, c, mc * 128 : (mc + 1) * 128],
                    ac_sb[:, c : c + 1],
                    start=(c == 0),
                    stop=(c == 3),
                )
        nc.vector.tensor_add(o_sb[:], o_ps[:], bo_sb[:])
        nc.sync.dma_start(out=out.rearrange("(c p) -> p c", p=128), in_=o_sb[:])

    nc.compile()
    return nc


def get_nc():
    if "nc" not in _cache:
        _cache["nc"] = _build()
    return _cache["nc"]


def host_prep(inputs: dict) -> list[dict]:
    """Per-core input maps: x slice + host-folded M + shared Wv/Wo/bo."""
    xs = np.ascontiguousarray(np.asarray(inputs["x"], dtype=np.float32))
    Wq = np.asarray(inputs["Wq"], dtype=np.float32)
    Wk = np.asarray(inputs["Wk"], dtype=np.float32)
    shared = {
        k: np.ascontiguousarray(np.asarray(inputs[k], dtype=np.float32))
        for k in ("Wv", "Wo", "bo")
    }
    in_maps = []
    for b in range(B):
        q_row = xs[b, -1] @ Wq                                   # [512]
        Mb = (Wk * q_row[None, :]).reshape(F, H, D).sum(-1)      # [256, 8]
        in_maps.append({"x": xs[b], "M": np.ascontiguousarray(Mb), **shared})
    return in_maps


def run_hw(inputs: dict) -> np.ndarray:
    nc = get_nc()
    res = run_bass_kernel_spmd(nc, host_prep(inputs), list(range(B)))
    return np.stack([res.results[b]["out"] for b in range(B)])


def kernel(**inputs) -> np.ndarray:
    return run_hw(inputs)


# revision 19
# speedup vs baseline: 1.1145x; 1.1145x over previous
"""Trainium2 Bass kernel: causal MHSA, last-position output (fp32, N-small matmuls).

Same collapsed math as kernel.py, but the two big matmuls are emitted in
transposed form so the streamed (free) dimension is 8 instead of 512/256 —
fp32 matmul cost scales with the free dim (4 cyc/row), while the 128-col
weight loads ride the separate LDW port:

    scores^T tiles [s,8] = xT_chunk.T @ M_chunk      (lhsT = xT, N=8)
    -> exp lands directly in the [s-part, h] layout the attention matmul
       needs, so the w-transpose stage disappears;
    attn^T chunks [f,8]  = x_chunk.T @ w_tile        (lhsT = x,  N=8)
    -> lands directly in the [f-part, h] layout the Wv matmul needs, so the
       attn_x transpose stage disappears.
    softmax sums via ones[128,1].T @ w_tiles accumulation (partition-dim sum).

Everything is fp32 end-to-end (no fp32r): HW rel err ~1.5e-6.
"""

import numpy as np
from contextlib import ExitStack

import concourse.bass as bass
import concourse.tile as tile
from concourse import bacc, mybir
from concourse.bass_utils import run_bass_kernel_spmd
from concourse.masks import make_identity

B, S, F, PROJ, H, D = 8, 2048, 256, 512, 8, 64
NT = S // 128        # 16 s-tiles
FC = F // 128        # 2 f-chunks
SG = 4               # s-tiles per pipeline group
NG = NT // SG        # 4 groups
f32 = mybir.dt.float32
EXP = mybir.ActivationFunctionType.Exp

_cache = {}


def _build():
    nc = bacc.Bacc("TRN2", target_bir_lowering=False, debug=False, num_devices=B)
    x = nc.dram_tensor("x", [S, F], f32, kind="ExternalInput").ap()
    M = nc.dram_tensor("M", [F, H], f32, kind="ExternalInput").ap()
    Wv = nc.dram_tensor("Wv", [F, PROJ], f32, kind="ExternalInput").ap()
    Wo = nc.dram_tensor("Wo", [PROJ, F], f32, kind="ExternalInput").ap()
    bo = nc.dram_tensor("bo", [F], f32, kind="ExternalInput").ap()
    # 0/1 selectors for the block-diag recip pattern: bd = A.T @ (B * recip)
    Abd = nc.dram_tensor("Abd", [H, 128], f32, kind="ExternalInput").ap()
    Bbd = nc.dram_tensor("Bbd", [H, 4], f32, kind="ExternalInput").ap()
    out = nc.dram_tensor("out", [F], f32, kind="ExternalOutput").ap()

    with tile.TileContext(nc) as tc, ExitStack() as ctx:
        P = ctx.enter_context(tc.tile_pool(name="persist", bufs=1))
        xtp = ctx.enter_context(tc.tile_pool(name="xtp", bufs=2, space="PSUM"))
        sct = ctx.enter_context(tc.tile_pool(name="sct", bufs=1, space="PSUM"))
        pers = ctx.enter_context(tc.tile_pool(name="pers", bufs=1, space="PSUM"))
        axp = ctx.enter_context(tc.tile_pool(name="axp", bufs=2, space="PSUM"))
        tailp = ctx.enter_context(tc.tile_pool(name="tailp", bufs=2, space="PSUM"))

        ident = P.tile([128, 128], f32)
        ones_col = P.tile([128, 1], f32)
        x_sb = P.tile([128, NT, F], f32)
        xT_sb = P.tile([128, FC, S], f32)
        m_sb = P.tile([128, FC, H], f32)
        wv_sb = P.tile([128, FC, PROJ], f32)
        wo_sb = P.tile([128, 4, F], f32)
        bo_sb = P.tile([128, FC], f32)
        wt_sb = P.tile([128, NT * H], f32)
        ssum_row = P.tile([1, H], f32)
        recip_row = P.tile([1, H], f32)
        srecip = P.tile([H, 1], f32)
        axT_sb = P.tile([128, FC * H], f32)
        abd_sb = P.tile([H, 128], f32)
        bbd_sb = P.tile([H, 4], f32)
        bw_sb = P.tile([H, 4], f32)
        bd_sb = P.tile([128, 4], f32)
        ac_sb = P.tile([128, 4], f32)
        o_sb = P.tile([128, FC], f32)
        dummy = P.tile([1, 1], f32)

        # trigger the ACT Exp table load early, overlapped with DMA
        nc.vector.memset(dummy[:], 0.0)
        nc.scalar.activation(out=dummy[:], in_=dummy[:], func=EXP)
        nc.vector.memset(ones_col[:], 1.0)

        make_identity(nc, ident[:])

        # ---- DMAs: x group 0 in halves (earlier compute start), rest of x,
        #      tiny M between, tail weights
        xr = x.rearrange("(t p) f -> p t f", p=128)
        nc.sync.dma_start(out=x_sb[:, 0:2, :], in_=xr[:, 0:2, :])
        nc.sync.dma_start(out=x_sb[:, 2:SG, :], in_=xr[:, 2:SG, :])
        nc.sync.dma_start(out=x_sb[:, SG : 2 * SG, :], in_=xr[:, SG : 2 * SG, :])
        nc.sync.dma_start(out=m_sb[:], in_=M.rearrange("(c p) h -> p c h", p=128))
        for g in range(2, NG):
            nc.sync.dma_start(
                out=x_sb[:, g * SG : (g + 1) * SG, :], in_=xr[:, g * SG : (g + 1) * SG, :]
            )
        nc.sync.dma_start(out=wv_sb[:], in_=Wv.rearrange("(c p) n -> p c n", p=128))
        nc.sync.dma_start(out=wo_sb[:], in_=Wo.rearrange("(c p) n -> p c n", p=128))
        nc.sync.dma_start(out=bo_sb[:], in_=bo.rearrange("(c p) -> p c", p=128))
        nc.sync.dma_start(out=abd_sb[:], in_=Abd[:])
        nc.sync.dma_start(out=bbd_sb[:], in_=Bbd[:])

        # ---- PE warm-up: open the HAM clock gate while DMA streams
        warm_ps = xtp.tile([128, SG * 128], f32, tag="xt")
        for j in range(8):
            nc.tensor.transpose(
                warm_ps[:, (j % SG) * 128 : (j % SG + 1) * 128], ident[:], ident[:]
            )

        # persistent PSUM accumulators
        sums_ps = pers.tile([1, SG * H], f32, tag="sums")
        axc_ps = [
            pers.tile([128, H], f32, tag=f"axc{c}", name=f"axc_ps{c}") for c in range(FC)
        ]

        # ---- pipelined per 512-row group
        for g in range(NG):
            lo = g * SG * 128
            for c in range(FC):
                xt_ps = xtp.tile([128, SG * 128], f32, tag="xt")
                for j in range(SG):
                    nc.tensor.transpose(
                        xt_ps[:, j * 128 : (j + 1) * 128],
                        x_sb[:, g * SG + j, c * 128 : (c + 1) * 128],
                        ident[:],
                    )
                nc.vector.tensor_copy(xT_sb[:, c, lo : lo + SG * 128], xt_ps[:])
            # scores^T tiles [128, 8] per s-tile, N=8 matmuls
            sct_ps = sct.tile([128, SG * H], f32, tag="sc")
            for j in range(SG):
                for c in range(FC):
                    nc.tensor.matmul(
                        sct_ps[:, j * H : (j + 1) * H],
                        xT_sb[:, c, lo + j * 128 : lo + (j + 1) * 128],
                        m_sb[:, c, :],
                        start=(c == 0),
                        stop=(c == FC - 1),
                    )
            # exp straight into the [s-part, h] layout attention needs
            nc.scalar.activation(
                out=wt_sb[:, g * SG * H : (g + 1) * SG * H],
                in_=sct_ps[:],
                func=EXP,
                scale=0.125,
            )
            # partition-dim softmax sums via ones-vector matmul
            nc.tensor.matmul(
                sums_ps[:],
                ones_col[:],
                wt_sb[:, g * SG * H : (g + 1) * SG * H],
                start=(g == 0),
                stop=(g == NG - 1),
                skip_group_check=True,
            )
            # attn^T chunks [f-part, h], N=8 matmuls, accumulate over all s-tiles
            for j in range(SG):
                t_idx = g * SG + j
                for c in range(FC):
                    nc.tensor.matmul(
                        axc_ps[c][:],
                        x_sb[:, t_idx, c * 128 : (c + 1) * 128],
                        wt_sb[:, t_idx * H : (t_idx + 1) * H],
                        start=(t_idx == 0),
                        stop=(t_idx == NT - 1),
                        skip_group_check=True,
                    )

        # ---- softmax denominator: [1, 4*8] partials -> [1, 8] -> recip -> [8, 1]
        sview = sums_ps[0:1, 0:1]
        sums_hj = bass.AP(
            tensor=sview.tensor, offset=sview.offset, ap=[sview.ap[0], [1, H], [H, SG]]
        )
        nc.vector.reduce_sum(out=ssum_row[:], in_=sums_hj, axis=mybir.AxisListType.X)
        nc.vector.reciprocal(recip_row[:], ssum_row[:])
        rt_ps = tailp.tile([H, 1], f32, tag="tail")
        nc.tensor.transpose(rt_ps[:], recip_row[:], ident[:1, :1])
        nc.vector.tensor_copy(srecip[:], rt_ps[:])

        # block-diag recip pattern bd[j, c] = recip[2c + (j>=64)] via one matmul
        nc.vector.tensor_scalar_mul(bw_sb[:], bbd_sb[:], srecip[:])
        bd_ps = tailp.tile([128, 4], f32, tag="tail")
        nc.tensor.matmul(bd_ps[:], abd_sb[:], bw_sb[:], start=True, stop=True)
        nc.vector.tensor_copy(bd_sb[:], bd_ps[:])

        # ---- attn^T to SBUF (already in [f-part, h] layout for the Wv matmul)
        for c in range(FC):
            nc.vector.tensor_copy(axT_sb[:, c * H : (c + 1) * H], axc_ps[c][:])

        # ---- attn_full^T blocks [p-part, h]: afT = Wv_block.T @ axT, N=8
        afT_ps = tailp.tile([128, 4 * H], f32, tag="tail")
        for pc in range(4):
            for c in range(FC):
                nc.tensor.matmul(
                    afT_ps[:, pc * H : (pc + 1) * H],
                    wv_sb[:, c, pc * 128 : (pc + 1) * 128],
                    axT_sb[:, c * H : (c + 1) * H],
                    start=(c == 0),
                    stop=(c == FC - 1),
                )
        # afT[j, 8pc+h] = attn_f[h, 128pc+j]; extract col 10c + (j>=64) per chunk,
        # normalizing by the block-diag recip pattern on the way out
        top = afT_ps[0:64, 0:1]
        bot = afT_ps[64:128, 1:2]
        nc.vector.tensor_mul(
            ac_sb[0:64, 0:4],
            bass.AP(tensor=top.tensor, offset=top.offset, ap=[top.ap[0], [10, 4]]),
            bd_sb[0:64, 0:4],
        )
        nc.vector.tensor_mul(
            ac_sb[64:128, 0:4],
            bass.AP(tensor=bot.tensor, offset=bot.offset, ap=[bot.ap[0], [10, 4]]),
            bd_sb[64:128, 0:4],
        )

        # ---- out[256] = attn_col.T @ Wo + bo  (column layout [128, 2])
        o_ps = tailp.tile([128, FC], f32, tag="tail")
        for mc in range(FC):
            for c in range(4):
                nc.tensor.matmul(
                    o_ps[:, mc : mc + 1],
                    wo_sb[:, c, mc * 128 : (mc + 1) * 128],
                    ac_sb[:, c : c + 1],
                    start=(c == 0),
                    stop=(c == 3),
                )
        nc.vector.tensor_add(o_sb[:], o_ps[:], bo_sb[:])
        nc.sync.dma_start(out=out.rearrange("(c p) -> p c", p=128), in_=o_sb[:])

    nc.compile()
    return nc


def get_nc():
    if "nc" not in _cache:
        _cache["nc"] = _build()
    return _cache["nc"]


def host_prep(inputs: dict) -> list[dict]:
    """Per-core input maps: x slice + host-folded M + shared Wv/Wo/bo."""
    xs = np.ascontiguousarray(np.asarray(inputs["x"], dtype=np.float32))
    Wq = np.asarray(inputs["Wq"], dtype=np.float32)
    Wk = np.asarray(inputs["Wk"], dtype=np.float32)
    shared = {
        k: np.ascontiguousarray(np.asarray(inputs[k], dtype=np.float32))
        for k in ("Wv", "Wo", "bo")
    }
    j = np.arange(128)
    h = np.arange(H)
    shared["Abd"] = np.ascontiguousarray(
        ((h[:, None] % 2) == (j[None, :] >= 64)).astype(np.float32)
    )
    shared["Bbd"] = np.ascontiguousarray(
        ((h[:, None] // 2) == np.arange(4)[None, :]).astype(np.float32)
    )
    in_maps = []
    for b in range(B):
        q_row = xs[b, -1] @ Wq                                   # [512]
        Mb = (Wk * q_row[None, :]).reshape(F, H, D).sum(-1)      # [256, 8]
        in_maps.append({"x": xs[b], "M": np.ascontiguousarray(Mb), **shared})
    return in_maps


def run_hw(inputs: dict) -> np.ndarray:
    nc = get_nc()
    res = run_bass_kernel_spmd(nc, host_prep(inputs), list(range(B)))
    return np.stack([res.results[b]["out"] for b in range(B)])


def kernel(**inputs) -> np.ndarray:
    return run_hw(inputs)


# revision 26
# speedup vs baseline: 1.1420x; 1.0247x over previous
"""Trainium2 Bass kernel: causal MHSA, last-position output (fp32, N-small matmuls).

Same collapsed math as kernel.py, but the two big matmuls are emitted in
transposed form so the streamed (free) dimension is 8 instead of 512/256 —
fp32 matmul cost scales with the free dim (4 cyc/row), while the 128-col
weight loads ride the separate LDW port:

    scores^T tiles [s,8] = xT_chunk.T @ M_chunk      (lhsT = xT, N=8)
    -> exp lands directly in the [s-part, h] layout the attention matmul
       needs, so the w-transpose stage disappears;
    attn^T chunks [f,8]  = x_chunk.T @ w_tile        (lhsT = x,  N=8)
    -> lands directly in the [f-part, h] layout the Wv matmul needs, so the
       attn_x transpose stage disappears.
    softmax sums via ones[128,1].T @ w_tiles accumulation (partition-dim sum).

Everything is fp32 end-to-end (no fp32r): HW rel err ~1.5e-6.
"""

import numpy as np
from contextlib import ExitStack

import concourse.bass as bass
import concourse.tile as tile
from concourse import bacc, mybir
from concourse.bass_utils import run_bass_kernel_spmd
from concourse.masks import make_identity

B, S, F, PROJ, H, D = 8, 2048, 256, 512, 8, 64
NT = S // 128        # 16 s-tiles
FC = F // 128        # 2 f-chunks
SG = 4               # s-tiles per pipeline group
NG = NT // SG        # 4 groups
f32 = mybir.dt.float32
EXP = mybir.ActivationFunctionType.Exp

_cache = {}


def _build():
    nc = bacc.Bacc("TRN2", target_bir_lowering=False, debug=False, num_devices=B)
    x = nc.dram_tensor("x", [S, F], f32, kind="ExternalInput").ap()
    M = nc.dram_tensor("M", [F, H], f32, kind="ExternalInput").ap()
    Wv = nc.dram_tensor("Wv", [F, PROJ], f32, kind="ExternalInput").ap()
    Wo = nc.dram_tensor("Wo", [PROJ, F], f32, kind="ExternalInput").ap()
    bo = nc.dram_tensor("bo", [FC, 128], f32, kind="ExternalInput").ap()
    # 0/1 selectors for the block-diag recip pattern: bd = A.T @ (B * recip)
    Abd = nc.dram_tensor("Abd", [H, 128], f32, kind="ExternalInput").ap()
    Bbd = nc.dram_tensor("Bbd", [H, 4], f32, kind="ExternalInput").ap()
    out = nc.dram_tensor("out", [F], f32, kind="ExternalOutput").ap()

    with tile.TileContext(nc) as tc, ExitStack() as ctx:
        P = ctx.enter_context(tc.tile_pool(name="persist", bufs=1))
        xtp = ctx.enter_context(tc.tile_pool(name="xtp", bufs=2, space="PSUM"))
        sct = ctx.enter_context(tc.tile_pool(name="sct", bufs=2, space="PSUM"))
        pers = ctx.enter_context(tc.tile_pool(name="pers", bufs=1, space="PSUM"))
        axp = ctx.enter_context(tc.tile_pool(name="axp", bufs=2, space="PSUM"))
        tailp = ctx.enter_context(tc.tile_pool(name="tailp", bufs=1, space="PSUM"))

        ident = P.tile([128, 128], f32)
        ones_col = P.tile([128, 1], f32)
        x_sb = P.tile([128, NT, F], f32)
        xT_sb = P.tile([128, FC, S], f32)
        m_sb = P.tile([128, FC, H], f32)
        wv_sb = P.tile([128, FC, PROJ], f32)
        wo_sb = P.tile([128, 4, F], f32)
        bo_sb = P.tile([1, FC, 128], f32)
        wt_sb = P.tile([128, NT * H], f32)
        srecip = P.tile([H, 1], f32)
        axT_sb = P.tile([128, FC * H], f32)
        abd_sb = P.tile([H, 128], f32)
        bbd_sb = P.tile([H, 4], f32)
        bw_sb = P.tile([H, 4], f32)
        bd_sb = P.tile([128, 4], f32)
        ac_sb = P.tile([128, 4], f32)
        o_sb = P.tile([128, FC], f32)
        dummy = P.tile([1, 1], f32)

        # trigger the ACT Exp table load early, overlapped with DMA
        nc.vector.memset(dummy[:], 0.0)
        nc.scalar.activation(out=dummy[:], in_=dummy[:], func=EXP)
        nc.vector.memset(ones_col[:], 1.0)

        make_identity(nc, ident[:])

        # ---- DMAs: x group 0 in halves (earlier compute start), rest of x,
        #      tiny M between, tail weights
        xr = x.rearrange("(t p) f -> p t f", p=128)
        nc.sync.dma_start(out=x_sb[:, 0:2, :], in_=xr[:, 0:2, :])
        nc.sync.dma_start(out=x_sb[:, 2:SG, :], in_=xr[:, 2:SG, :])
        nc.sync.dma_start(out=x_sb[:, SG : 2 * SG, :], in_=xr[:, SG : 2 * SG, :])
        nc.sync.dma_start(out=m_sb[:], in_=M.rearrange("(c p) h -> p c h", p=128))
        for g in range(2, NG):
            nc.sync.dma_start(
                out=x_sb[:, g * SG : (g + 1) * SG, :], in_=xr[:, g * SG : (g + 1) * SG, :]
            )
        nc.sync.dma_start(out=wv_sb[:], in_=Wv.rearrange("(c p) n -> p c n", p=128))
        nc.sync.dma_start(out=wo_sb[:], in_=Wo.rearrange("(c p) n -> p c n", p=128))
        nc.sync.dma_start(out=bo_sb[0:1, :, :], in_=bo[:])
        nc.sync.dma_start(out=abd_sb[:], in_=Abd[:])
        nc.sync.dma_start(out=bbd_sb[:], in_=Bbd[:])

        # ---- PE warm-up: open the HAM clock gate while DMA streams
        warm_ps = xtp.tile([128, SG * 128], f32, tag="xt")
        for j in range(8):
            nc.tensor.transpose(
                warm_ps[:, (j % SG) * 128 : (j % SG + 1) * 128], ident[:], ident[:]
            )

        # persistent PSUM accumulators
        sums_ps = pers.tile([H, 1], f32, tag="sums")
        axc_ps = [
            pers.tile([128, H], f32, tag=f"axc{c}", name=f"axc_ps{c}") for c in range(FC)
        ]

        # ---- pipelined per 512-row group
        for g in range(NG):
            lo = g * SG * 128
            for c in range(FC):
                xt_ps = xtp.tile([128, SG * 128], f32, tag="xt")
                for j in range(SG):
                    nc.tensor.transpose(
                        xt_ps[:, j * 128 : (j + 1) * 128],
                        x_sb[:, g * SG + j, c * 128 : (c + 1) * 128],
                        ident[:],
                    )
                nc.vector.tensor_copy(xT_sb[:, c, lo : lo + SG * 128], xt_ps[:])
            # scores^T tiles [128, 8] per s-tile, N=8 matmuls
            sct_ps = sct.tile([128, SG * H], f32, tag="sc")
            for j in range(SG):
                for c in range(FC):
                    nc.tensor.matmul(
                        sct_ps[:, j * H : (j + 1) * H],
                        xT_sb[:, c, lo + j * 128 : lo + (j + 1) * 128],
                        m_sb[:, c, :],
                        start=(c == 0),
                        stop=(c == FC - 1),
                    )
            # exp straight into the [s-part, h] layout attention needs
            nc.scalar.activation(
                out=wt_sb[:, g * SG * H : (g + 1) * SG * H],
                in_=sct_ps[:],
                func=EXP,
                scale=0.125,
            )
            # attn^T chunks [f-part, h] and softmax sums column, accumulated
            for j in range(SG):
                t_idx = g * SG + j
                nc.tensor.matmul(
                    sums_ps[:],
                    wt_sb[:, t_idx * H : (t_idx + 1) * H],
                    ones_col[:],
                    start=(t_idx == 0),
                    stop=(t_idx == NT - 1),
                    skip_group_check=True,
                )
                for c in range(FC):
                    nc.tensor.matmul(
                        axc_ps[c][:],
                        x_sb[:, t_idx, c * 128 : (c + 1) * 128],
                        wt_sb[:, t_idx * H : (t_idx + 1) * H],
                        start=(t_idx == 0),
                        stop=(t_idx == NT - 1),
                        skip_group_check=True,
                    )

        # ---- softmax denominator: reciprocal straight off the PSUM column
        nc.vector.reciprocal(srecip[:], sums_ps[:])

        # block-diag recip pattern bd[j, c] = recip[2c + (j>=64)] via one matmul
        nc.vector.tensor_scalar_mul(bw_sb[:], bbd_sb[:], srecip[:])
        bd_ps = tailp.tile([128, 4], f32, tag="tail")
        nc.tensor.matmul(bd_ps[:], abd_sb[:], bw_sb[:], start=True, stop=True)
        nc.vector.tensor_copy(bd_sb[:], bd_ps[:])

        # ---- attn^T to SBUF (already in [f-part, h] layout for the Wv matmul)
        for c in range(FC):
            nc.vector.tensor_copy(axT_sb[:, c * H : (c + 1) * H], axc_ps[c][:])

        # ---- attn_full^T blocks [p-part, h]: afT = Wv_block.T @ axT, N=8
        afT_ps = xtp.tile([128, 4 * H], f32, tag="xt")
        for pc in range(4):
            for c in range(FC):
                nc.tensor.matmul(
                    afT_ps[:, pc * H : (pc + 1) * H],
                    wv_sb[:, c, pc * 128 : (pc + 1) * 128],
                    axT_sb[:, c * H : (c + 1) * H],
                    start=(c == 0),
                    stop=(c == FC - 1),
                )
        # afT[j, 8pc+h] = attn_f[h, 128pc+j]; extract col 10c + (j>=64) per chunk,
        # normalizing by the block-diag recip pattern on the way out
        top = afT_ps[0:64, 0:1]
        bot = afT_ps[64:128, 1:2]
        nc.vector.tensor_mul(
            ac_sb[0:64, 0:4],
            bass.AP(tensor=top.tensor, offset=top.offset, ap=[top.ap[0], [10, 4]]),
            bd_sb[0:64, 0:4],
        )
        nc.vector.tensor_mul(
            ac_sb[64:128, 0:4],
            bass.AP(tensor=bot.tensor, offset=bot.offset, ap=[bot.ap[0], [10, 4]]),
            bd_sb[64:128, 0:4],
        )

        # ---- out[256] = attn_col.T @ Wo + bo  (column layout [128, 2]);
        #      bias enters as a rank-1 accumulation, result DMAs out of PSUM
        o_ps = tailp.tile([128, FC], f32, tag="tail")
        for mc in range(FC):
            for c in range(4):
                nc.tensor.matmul(
                    o_ps[:, mc : mc + 1],
                    wo_sb[:, c, mc * 128 : (mc + 1) * 128],
                    ac_sb[:, c : c + 1],
                    start=(c == 0),
                    stop=False,
                    skip_group_check=True,
                )
            nc.tensor.matmul(
                o_ps[:, mc : mc + 1],
                bo_sb[0:1, mc, :],
                ones_col[0:1, 0:1],
                start=False,
                stop=True,
                skip_group_check=True,
            )
        nc.vector.tensor_copy(o_sb[:], o_ps[:])
        nc.sync.dma_start(out=out.rearrange("(c p) -> p c", p=128), in_=o_sb[:])

    nc.compile()
    return nc


def get_nc():
    if "nc" not in _cache:
        _cache["nc"] = _build()
    return _cache["nc"]


def host_prep(inputs: dict) -> list[dict]:
    """Per-core input maps: x slice + host-folded M + shared Wv/Wo/bo."""
    xs = np.ascontiguousarray(np.asarray(inputs["x"], dtype=np.float32))
    Wq = np.asarray(inputs["Wq"], dtype=np.float32)
    Wk = np.asarray(inputs["Wk"], dtype=np.float32)
    shared = {
        k: np.ascontiguousarray(np.asarray(inputs[k], dtype=np.float32))
        for k in ("Wv", "Wo")
    }
    shared["bo"] = np.ascontiguousarray(
        np.asarray(inputs["bo"], dtype=np.float32).reshape(FC, 128)
    )
    j = np.arange(128)
    h = np.arange(H)
    shared["Abd"] = np.ascontiguousarray(
        ((h[:, None] % 2) == (j[None, :] >= 64)).astype(np.float32)
    )
    shared["Bbd"] = np.ascontiguousarray(
        ((h[:, None] // 2) == np.arange(4)[None, :]).astype(np.float32)
    )
    in_maps = []
    for b in range(B):
        q_row = xs[b, -1] @ Wq                                   # [512]
        Mb = (Wk * q_row[None, :]).reshape(F, H, D).sum(-1)      # [256, 8]
        in_maps.append({"x": xs[b], "M": np.ascontiguousarray(Mb), **shared})
    return in_maps


def run_hw(inputs: dict) -> np.ndarray:
    nc = get_nc()
    res = run_bass_kernel_spmd(nc, host_prep(inputs), list(range(B)))
    return np.stack([res.results[b]["out"] for b in range(B)])


def kernel(**inputs) -> np.ndarray:
    return run_hw(inputs)


# revision 27
# speedup vs baseline: 1.1444x; 1.0020x over previous
"""Trainium2 Bass kernel: causal MHSA, last-position output (fp32, N-small matmuls).

Same collapsed math as kernel.py, but the two big matmuls are emitted in
transposed form so the streamed (free) dimension is 8 instead of 512/256 —
fp32 matmul cost scales with the free dim (4 cyc/row), while the 128-col
weight loads ride the separate LDW port:

    scores^T tiles [s,8] = xT_chunk.T @ M_chunk      (lhsT = xT, N=8)
    -> exp lands directly in the [s-part, h] layout the attention matmul
       needs, so the w-transpose stage disappears;
    attn^T chunks [f,8]  = x_chunk.T @ w_tile        (lhsT = x,  N=8)
    -> lands directly in the [f-part, h] layout the Wv matmul needs, so the
       attn_x transpose stage disappears.
    softmax sums via ones[128,1].T @ w_tiles accumulation (partition-dim sum).

Everything is fp32 end-to-end (no fp32r): HW rel err ~1.5e-6.
"""

import numpy as np
from contextlib import ExitStack

import concourse.bass as bass
import concourse.tile as tile
from concourse import bacc, mybir
from concourse.bass_utils import run_bass_kernel_spmd
from concourse.masks import make_identity

B, S, F, PROJ, H, D = 8, 2048, 256, 512, 8, 64
NT = S // 128        # 16 s-tiles
FC = F // 128        # 2 f-chunks
SG = 4               # s-tiles per pipeline group
NG = NT // SG        # 4 groups
f32 = mybir.dt.float32
EXP = mybir.ActivationFunctionType.Exp

_cache = {}


def _build():
    nc = bacc.Bacc("TRN2", target_bir_lowering=False, debug=False, num_devices=B)
    x = nc.dram_tensor("x", [S, F], f32, kind="ExternalInput").ap()
    M = nc.dram_tensor("M", [F, H], f32, kind="ExternalInput").ap()
    Wv = nc.dram_tensor("Wv", [F, PROJ], f32, kind="ExternalInput").ap()
    Wo = nc.dram_tensor("Wo", [PROJ, F], f32, kind="ExternalInput").ap()
    bo = nc.dram_tensor("bo", [FC, 128], f32, kind="ExternalInput").ap()
    # 0/1 selectors for the block-diag recip pattern: bd = A.T @ (B * recip)
    Abd = nc.dram_tensor("Abd", [H, 128], f32, kind="ExternalInput").ap()
    Bbd = nc.dram_tensor("Bbd", [H, 4], f32, kind="ExternalInput").ap()
    out = nc.dram_tensor("out", [F], f32, kind="ExternalOutput").ap()

    with tile.TileContext(nc) as tc, ExitStack() as ctx:
        P = ctx.enter_context(tc.tile_pool(name="persist", bufs=1))
        xtp = ctx.enter_context(tc.tile_pool(name="xtp", bufs=3, space="PSUM"))
        sct = ctx.enter_context(tc.tile_pool(name="sct", bufs=1, space="PSUM"))
        pers = ctx.enter_context(tc.tile_pool(name="pers", bufs=1, space="PSUM"))
        axp = ctx.enter_context(tc.tile_pool(name="axp", bufs=2, space="PSUM"))
        tailp = ctx.enter_context(tc.tile_pool(name="tailp", bufs=1, space="PSUM"))

        ident = P.tile([128, 128], f32)
        ones_col = P.tile([128, 1], f32)
        x_sb = P.tile([128, NT, F], f32)
        xT_sb = P.tile([128, FC, S], f32)
        m_sb = P.tile([128, FC, H], f32)
        wv_sb = P.tile([128, FC, PROJ], f32)
        wo_sb = P.tile([128, 4, F], f32)
        bo_sb = P.tile([1, FC, 128], f32)
        wt_sb = P.tile([128, NT * H], f32)
        srecip = P.tile([H, 1], f32)
        axT_sb = P.tile([128, FC * H], f32)
        abd_sb = P.tile([H, 128], f32)
        bbd_sb = P.tile([H, 4], f32)
        bw_sb = P.tile([H, 4], f32)
        bd_sb = P.tile([128, 4], f32)
        ac_sb = P.tile([128, 4], f32)
        o_sb = P.tile([128, FC], f32)
        dummy = P.tile([1, 1], f32)

        # trigger the ACT Exp table load early, overlapped with DMA
        nc.vector.memset(dummy[:], 0.0)
        nc.scalar.activation(out=dummy[:], in_=dummy[:], func=EXP)
        nc.vector.memset(ones_col[:], 1.0)

        make_identity(nc, ident[:])

        # ---- DMAs: x group 0 in halves (earlier compute start), rest of x,
        #      tiny M between, tail weights
        xr = x.rearrange("(t p) f -> p t f", p=128)
        nc.sync.dma_start(out=x_sb[:, 0:2, :], in_=xr[:, 0:2, :])
        nc.sync.dma_start(out=x_sb[:, 2:SG, :], in_=xr[:, 2:SG, :])
        nc.sync.dma_start(out=x_sb[:, SG : 2 * SG, :], in_=xr[:, SG : 2 * SG, :])
        nc.sync.dma_start(out=m_sb[:], in_=M.rearrange("(c p) h -> p c h", p=128))
        for g in range(2, NG):
            nc.sync.dma_start(
                out=x_sb[:, g * SG : (g + 1) * SG, :], in_=xr[:, g * SG : (g + 1) * SG, :]
            )
        nc.sync.dma_start(out=wv_sb[:], in_=Wv.rearrange("(c p) n -> p c n", p=128))
        nc.sync.dma_start(out=wo_sb[:], in_=Wo.rearrange("(c p) n -> p c n", p=128))
        nc.sync.dma_start(out=bo_sb[0:1, :, :], in_=bo[:])
        nc.sync.dma_start(out=abd_sb[:], in_=Abd[:])
        nc.sync.dma_start(out=bbd_sb[:], in_=Bbd[:])

        # ---- PE warm-up: open the HAM clock gate while DMA streams
        warm_ps = xtp.tile([128, SG * 128], f32, tag="xt")
        for j in range(8):
            nc.tensor.transpose(
                warm_ps[:, (j % SG) * 128 : (j % SG + 1) * 128], ident[:], ident[:]
            )

        # persistent PSUM accumulators
        sums_ps = pers.tile([H, 1], f32, tag="sums")
        axc_ps = [
            pers.tile([128, H], f32, tag=f"axc{c}", name=f"axc_ps{c}") for c in range(FC)
        ]

        # ---- pipelined per 512-row group
        for g in range(NG):
            lo = g * SG * 128
            for c in range(FC):
                xt_ps = xtp.tile([128, SG * 128], f32, tag="xt")
                for j in range(SG):
                    nc.tensor.transpose(
                        xt_ps[:, j * 128 : (j + 1) * 128],
                        x_sb[:, g * SG + j, c * 128 : (c + 1) * 128],
                        ident[:],
                    )
                nc.vector.tensor_copy(xT_sb[:, c, lo : lo + SG * 128], xt_ps[:])
            # scores^T tiles [128, 8] per s-tile, N=8 matmuls
            sct_ps = sct.tile([128, SG * H], f32, tag="sc")
            for j in range(SG):
                for c in range(FC):
                    nc.tensor.matmul(
                        sct_ps[:, j * H : (j + 1) * H],
                        xT_sb[:, c, lo + j * 128 : lo + (j + 1) * 128],
                        m_sb[:, c, :],
                        start=(c == 0),
                        stop=(c == FC - 1),
                    )
            # exp straight into the [s-part, h] layout attention needs
            nc.scalar.activation(
                out=wt_sb[:, g * SG * H : (g + 1) * SG * H],
                in_=sct_ps[:],
                func=EXP,
                scale=0.125,
            )
            # attn^T chunks [f-part, h] and softmax sums column, accumulated
            for j in range(SG):
                t_idx = g * SG + j
                nc.tensor.matmul(
                    sums_ps[:],
                    wt_sb[:, t_idx * H : (t_idx + 1) * H],
                    ones_col[:],
                    start=(t_idx == 0),
                    stop=(t_idx == NT - 1),
                    skip_group_check=True,
                )
                for c in range(FC):
                    nc.tensor.matmul(
                        axc_ps[c][:],
                        x_sb[:, t_idx, c * 128 : (c + 1) * 128],
                        wt_sb[:, t_idx * H : (t_idx + 1) * H],
                        start=(t_idx == 0),
                        stop=(t_idx == NT - 1),
                        skip_group_check=True,
                    )

        # ---- softmax denominator: reciprocal straight off the PSUM column
        nc.vector.reciprocal(srecip[:], sums_ps[:])

        # block-diag recip pattern bd[j, c] = recip[2c + (j>=64)] via one matmul
        nc.vector.tensor_scalar_mul(bw_sb[:], bbd_sb[:], srecip[:])
        bd_ps = tailp.tile([128, 4], f32, tag="tail")
        nc.tensor.matmul(bd_ps[:], abd_sb[:], bw_sb[:], start=True, stop=True)
        nc.vector.tensor_copy(bd_sb[:], bd_ps[:])

        # ---- attn^T to SBUF (already in [f-part, h] layout for the Wv matmul)
        for c in range(FC):
            nc.vector.tensor_copy(axT_sb[:, c * H : (c + 1) * H], axc_ps[c][:])

        # ---- attn_full^T blocks [p-part, h]: afT = Wv_block.T @ axT, N=8
        afT_ps = xtp.tile([128, 4 * H], f32, tag="xt")
        for pc in range(4):
            for c in range(FC):
                nc.tensor.matmul(
                    afT_ps[:, pc * H : (pc + 1) * H],
                    wv_sb[:, c, pc * 128 : (pc + 1) * 128],
                    axT_sb[:, c * H : (c + 1) * H],
                    start=(c == 0),
                    stop=(c == FC - 1),
                )
        # afT[j, 8pc+h] = attn_f[h, 128pc+j]; extract col 10c + (j>=64) per chunk,
        # normalizing by the block-diag recip pattern on the way out
        top = afT_ps[0:64, 0:1]
        bot = afT_ps[64:128, 1:2]
        nc.vector.tensor_mul(
            ac_sb[0:64, 0:4],
            bass.AP(tensor=top.tensor, offset=top.offset, ap=[top.ap[0], [10, 4]]),
            bd_sb[0:64, 0:4],
        )
        nc.vector.tensor_mul(
            ac_sb[64:128, 0:4],
            bass.AP(tensor=bot.tensor, offset=bot.offset, ap=[bot.ap[0], [10, 4]]),
            bd_sb[64:128, 0:4],
        )

        # ---- out[256] = attn_col.T @ Wo + bo  (column layout [128, 2]);
        #      bias enters as a rank-1 accumulation, result DMAs out of PSUM
        o_ps = tailp.tile([128, FC], f32, tag="tail")
        for mc in range(FC):
            for c in range(4):
                nc.tensor.matmul(
                    o_ps[:, mc : mc + 1],
                    wo_sb[:, c, mc * 128 : (mc + 1) * 128],
                    ac_sb[:, c : c + 1],
                    start=(c == 0),
                    stop=False,
                    skip_group_check=True,
                )
            nc.tensor.matmul(
                o_ps[:, mc : mc + 1],
                bo_sb[0:1, mc, :],
                ones_col[0:1, 0:1],
                start=False,
                stop=True,
                skip_group_check=True,
            )
        nc.vector.tensor_copy(o_sb[:], o_ps[:])
        nc.sync.dma_start(out=out.rearrange("(c p) -> p c", p=128), in_=o_sb[:])

    nc.compile()
    return nc


def get_nc():
    if "nc" not in _cache:
        _cache["nc"] = _build()
    return _cache["nc"]


def host_prep(inputs: dict) -> list[dict]:
    """Per-core input maps: x slice + host-folded M + shared Wv/Wo/bo."""
    xs = np.ascontiguousarray(np.asarray(inputs["x"], dtype=np.float32))
    Wq = np.asarray(inputs["Wq"], dtype=np.float32)
    Wk = np.asarray(inputs["Wk"], dtype=np.float32)
    shared = {
        k: np.ascontiguousarray(np.asarray(inputs[k], dtype=np.float32))
        for k in ("Wv", "Wo")
    }
    shared["bo"] = np.ascontiguousarray(
        np.asarray(inputs["bo"], dtype=np.float32).reshape(FC, 128)
    )
    j = np.arange(128)
    h = np.arange(H)
    shared["Abd"] = np.ascontiguousarray(
        ((h[:, None] % 2) == (j[None, :] >= 64)).astype(np.float32)
    )
    shared["Bbd"] = np.ascontiguousarray(
        ((h[:, None] // 2) == np.arange(4)[None, :]).astype(np.float32)
    )
    in_maps = []
    for b in range(B):
        q_row = xs[b, -1] @ Wq                                   # [512]
        Mb = (Wk * q_row[None, :]).reshape(F, H, D).sum(-1)      # [256, 8]
        in_maps.append({"x": xs[b], "M": np.ascontiguousarray(Mb), **shared})
    return in_maps


def run_hw(inputs: dict) -> np.ndarray:
    nc = get_nc()
    res = run_bass_kernel_spmd(nc, host_prep(inputs), list(range(B)))
    return np.stack([res.results[b]["out"] for b in range(B)])


def kernel(**inputs) -> np.ndarray:
    return run_hw(inputs)
